# revision 29
# baseline (speedup 1.0000x reference)
"""Trainium2 Bass kernel for causal multi-head attention.

Problem: B=2, S=2048, D=1024, H=16 heads (head_dim=64), fp32.
  y = softmax(causal(x@wq @ (x@wk)^T / sqrt(64))) @ (x@wv) @ wo + bo

Sharding (8 NeuronCores): 2 batches x 4 head-groups (4 heads each).
Each core computes, for its batch b and its 4 heads:
  - Q^T, K^T in [j, t] layout and V in [t, j] layout (j = 256 head cols)
  - scores^T[k, q] = K^T.T-free matmul, exp (scale=1/8, no max-sub --
    scores are ~N(0,1) so fp32 exp is safe), causal mask, then
    ctx^T[hd, q] with an appended ones-column giving softmax sums for free
  - per-q normalization via reciprocal + gpsimd partition-broadcast
  - partial y = ctx^T.T @ wo_slice  (row-shard of wo)
Host sums the 4 partials per batch and adds bo.

Matmul operands are bf16 (host-cast); accumulation is fp32 in PSUM.
"""

import numpy as np

B, S, D, H = 2, 2048, 1024, 16
HD = 64          # head dim
NCORES = 8
HG = 4           # heads per core
JW = HG * HD     # 256: per-core head columns
P = 128
DC = D // P      # 8 contraction chunks for projections
TCB = S // 512   # 4: 512-token blocks
NT = S // P      # 16: 128-token chunks

_CACHE = {}


def _build_nc(debug=False):
    import concourse.tile as tile
    from concourse import bacc, mybir

    f32 = mybir.dt.float32
    f32r = mybir.dt.bfloat16  # matmul operand dtype (fp32 accum in PSUM)
    EXP = mybir.ActivationFunctionType.Exp

    nc = bacc.Bacc(None, target_bir_lowering=False)

    xT_h = nc.dram_tensor("xT", [D, S], f32r, kind="ExternalInput")
    wq_h = nc.dram_tensor("wq", [D, JW], f32r, kind="ExternalInput")
    wk_h = nc.dram_tensor("wk", [D, JW], f32r, kind="ExternalInput")
    wv_h = nc.dram_tensor("wv", [D, JW], f32r, kind="ExternalInput")
    wo_h = nc.dram_tensor("wo", [JW, D], f32r, kind="ExternalInput")
    tri_h = nc.dram_tensor("tri", [P, P], f32r, kind="ExternalInput")
    vones_h = nc.dram_tensor("vones", [P, NT * HG], f32r, kind="ExternalInput")
    y_h = nc.dram_tensor("y", [S, D], f32, kind="ExternalOutput")
    if debug:
        dQT_h = nc.dram_tensor("dQT", [P, 2, S], f32r, kind="ExternalOutput")
        dKT_h = nc.dram_tensor("dKT", [P, 2, S], f32r, kind="ExternalOutput")
        dVg_h = nc.dram_tensor("dVg", [P, NT, HG * (HD + 2)], f32r,
                               kind="ExternalOutput")
        dctxn_h = nc.dram_tensor("dctxn", [P, 2, S], f32r, kind="ExternalOutput")
        dpc_h = nc.dram_tensor("dpc", [2, HD + 1, 512], f32, kind="ExternalOutput")
        drt_h = nc.dram_tensor("drt", [2, 512], f32, kind="ExternalOutput")
        drbc_h = nc.dram_tensor("drbc", [2, HD, 512], f32, kind="ExternalOutput")
        det_h = nc.dram_tensor("det", [4, P, 512], f32r, kind="ExternalOutput")

    with tile.TileContext(nc) as tc:
        with (
            tc.tile_pool(name="const", bufs=1) as cp,
            tc.tile_pool(name="work", bufs=2) as wp,
            tc.tile_pool(name="psum", bufs=2, space="PSUM") as pp,
        ):
            # ---- resident SBUF tensors ----
            xT_sb = cp.tile([P, DC, S], f32r, name="xT_sb")          # 64KB/part
            wq_sb = cp.tile([P, DC, JW], f32r, name="wq_sb")         # 8KB
            wk_sb = cp.tile([P, DC, JW], f32r, name="wk_sb")
            wv_sb = cp.tile([P, DC, JW], f32r, name="wv_sb")
            wo_sb = cp.tile([P, 2, D], f32r, name="wo_sb")           # 8KB
            tri_sb = cp.tile([P, P], f32r, name="tri_sb")
            QT = cp.tile([P, 2, S], f32r, name="QT")                 # 16KB
            KT = cp.tile([P, 2, S], f32r, name="KT")
            Vg = cp.tile([P, NT, HG * (HD + 2)], f32r, name="Vg")
            ctxn = cp.tile([P, 2, S], f32r, name="ctxn")             # 16KB

            # ---- input DMAs (small weights first so PE can start) ----
            nc.sync.dma_start(
                out=wk_sb, in_=wk_h[:].rearrange("(dc p) j -> p dc j", p=P)
            )
            nc.sync.dma_start(
                out=wq_sb, in_=wq_h[:].rearrange("(dc p) j -> p dc j", p=P)
            )
            for dc in range(DC):
                nc.sync.dma_start(
                    out=xT_sb[:, dc, :], in_=xT_h[dc * P:(dc + 1) * P, :]
                )
            nc.sync.dma_start(
                out=wv_sb, in_=wv_h[:].rearrange("(dc p) j -> p dc j", p=P)
            )
            nc.sync.dma_start(
                out=wo_sb, in_=wo_h[:].rearrange("(ch p) n -> p ch n", p=P)
            )
            nc.sync.dma_start(out=tri_sb, in_=tri_h[:, :])
            # ones columns of Vg (V writes fill the rest; scalar ring so it
            # lands before the first ctx matmul, not behind the xT transfer)
            nc.scalar.dma_start(
                out=Vg[:, :, :].rearrange("p t (h c) -> p t h c", c=HD + 2)[:, :, :, HD],
                in_=vones_h[:].rearrange("p (t h) -> p t h", h=HG),
            )

            # ---- fused per-q-block pipeline:
            #      project this block's Q/K/V stripe, then attention ----
            for j4 in range(TCB):          # q-block [512*j4, 512*j4+512)
                tb = j4
                for w_sb, dst in ((wk_sb, KT), (wq_sb, QT)):
                    for jc in range(2):
                        pq = pp.tile([P, 512], f32, tag="mm", bufs=2, name="pq")
                        for dc in range(DC):
                            nc.tensor.matmul(
                                pq,
                                lhsT=w_sb[:, dc, jc * P:(jc + 1) * P],
                                rhs=xT_sb[:, dc, tb * 512:(tb + 1) * 512],
                                start=(dc == 0),
                                stop=(dc == DC - 1),
                            )
                        nc.vector.tensor_copy(
                            out=dst[:, jc, tb * 512:(tb + 1) * 512], in_=pq
                        )
                for tv in range(4 * j4, 4 * j4 + 4):
                    pv = pp.tile([P, JW], f32, tag="pcx", bufs=4, name="pv")
                    for dc in range(DC):
                        nc.tensor.matmul(
                            pv,
                            lhsT=xT_sb[:, dc, tv * P:(tv + 1) * P],
                            rhs=wv_sb[:, dc, :],
                            start=(dc == 0),
                            stop=(dc == DC - 1),
                        )
                    nc.vector.tensor_copy(
                        out=Vg[:, tv, :].rearrange("p (h c) -> p h c", c=HD + 2)[:, :, 0:HD],
                        in_=pv.rearrange("p (h c) -> p h c", c=HD),
                    )
                for pr in range(2):        # pair index = chunk index (h//2)
                    qs = slice(j4 * 512, (j4 + 1) * 512)
                    nchunks = 4 * j4 + 4
                    pctx = []
                    for hh in range(2):
                        pc = pp.tile([HD + 1, 512], f32, tag="pcx", bufs=4,
                                     name=f"pc{hh}")
                        pctx.append(pc)
                    for c in range(nchunks):
                        # columns [0, o) are fully masked for this k-chunk:
                        # skip them in scores, exp and ctx entirely.
                        o = P * (c - 4 * j4) if c >= 4 * j4 else 0
                        ps2 = pp.tile([P, 2, 512], f32, tag="mm", bufs=2,
                                      name="ps2")
                        for hh in range(2):
                            bp = HD * hh   # partition base for this head
                            nc.tensor.matmul(
                                ps2[:, hh, o:512],
                                lhsT=KT[bp:bp + HD, pr, c * P:(c + 1) * P],
                                rhs=QT[bp:bp + HD, pr,
                                       j4 * 512 + o:(j4 + 1) * 512],
                                start=True,
                                stop=True,
                            )
                        et = wp.tile([P, 2, 512], f32r, tag="exp", bufs=8,
                                     name="et")
                        nc.scalar.activation(
                            out=et[:, :, o:512], in_=ps2[:, :, o:512],
                            func=EXP, scale=0.125,
                        )
                        if c >= 4 * j4:
                            nc.vector.tensor_mul(
                                out=et[:, :, o:o + P],
                                in0=et[:, :, o:o + P],
                                in1=tri_sb[:, None, :].to_broadcast([P, 2, P]),
                            )
                        if debug and pr == 0 and j4 == 0:
                            nc.sync.dma_start(out=det_h[c], in_=et[:, 0, :])
                        for hh in range(2):
                            h = 2 * pr + hh
                            nc.tensor.matmul(
                                pctx[hh][:, o:512],
                                lhsT=Vg[:, c, h * (HD + 2):h * (HD + 2) + HD + 1],
                                rhs=et[:, hh, o:512],
                                start=(c == 0),
                                stop=(c == nchunks - 1),
                            )
                    # evacuate ctx psum early (frees the bank), then
                    # normalize: ctx^T[hd, q] * (1/sum[q])
                    for hh in range(2):
                        pc = pctx[hh]
                        # HW quirk: custom-DVE + partition_broadcast misread
                        # sources at partition base 64 -- hop sums to base 0.
                        sums = wp.tile([1, 512], f32, tag="sums", bufs=4,
                                       name="sums")
                        nc.vector.tensor_copy(out=sums, in_=pc[HD:HD + 1, :])
                        if debug and pr == 0 and j4 == 0:
                            dtmp = wp.tile([HD, 512], f32, tag="dtmp",
                                           bufs=2, name="dtmp")
                            nc.vector.tensor_copy(out=dtmp, in_=pc[0:HD, :])
                            nc.sync.dma_start(out=dpc_h[hh, 0:HD, :], in_=dtmp)
                            nc.sync.dma_start(out=dpc_h[hh, HD:HD + 1, :],
                                              in_=sums)
                        rt = wp.tile([1, 512], f32, tag="rt", bufs=4,
                                     name="rt")
                        nc.vector.reciprocal_approx_fast(out=rt, in_=sums)
                        rbc = wp.tile([HD, 512], f32, tag="rbc", bufs=4,
                                      name="rbc")
                        nc.gpsimd.partition_broadcast(
                            rbc[:, :], rt[0:1, :], channels=HD
                        )
                        if debug and pr == 0 and j4 == 0:
                            nc.sync.dma_start(out=drt_h[hh:hh + 1, :],
                                              in_=rt[0:1, :])
                            nc.sync.dma_start(out=drbc_h[hh], in_=rbc[:, :])
                        nc.vector.tensor_mul(
                            out=ctxn[HD * hh:HD * (hh + 1), pr, qs],
                            in0=pc[0:HD, :],
                            in1=rbc[:, :],
                        )

                # ---- output projection for this q-block ----
                for tb4 in range(4):
                    tb = 4 * j4 + tb4
                    ysb = wp.tile([P, D], f32, tag="y", bufs=3, name="ysb")
                    for nn in range(2):
                        py = pp.tile([P, 512], f32, tag="pcx", bufs=4,
                                     name="py")
                        for jc in range(2):
                            nc.tensor.matmul(
                                py,
                                lhsT=ctxn[:, jc, tb * P:(tb + 1) * P],
                                rhs=wo_sb[:, jc, nn * 512:(nn + 1) * 512],
                                start=(jc == 0),
                                stop=(jc == 1),
                            )
                        nc.scalar.copy(
                            out=ysb[:, nn * 512:(nn + 1) * 512], in_=py
                        )
                    nc.sync.dma_start(
                        out=y_h[tb * P:(tb + 1) * P, :], in_=ysb
                    )

            if debug:
                nc.sync.dma_start(out=dQT_h[:], in_=QT[:, :, :])
                nc.sync.dma_start(out=dKT_h[:], in_=KT[:, :, :])
                nc.sync.dma_start(out=dVg_h[:], in_=Vg[:, :, :])
                nc.sync.dma_start(out=dctxn_h[:], in_=ctxn[:, :, :])

    nc.compile()
    return nc


def get_nc(debug=False):
    key = ("nc", debug)
    if key not in _CACHE:
        _CACHE[key] = _build_nc(debug=debug)
    return _CACHE[key]


def make_in_maps(x, wq, wk, wv, wo):
    import ml_dtypes
    bf16 = ml_dtypes.bfloat16
    x = np.asarray(x, dtype=np.float32)
    wq = np.asarray(wq, dtype=np.float32).astype(bf16)
    wk = np.asarray(wk, dtype=np.float32).astype(bf16)
    wv = np.asarray(wv, dtype=np.float32).astype(bf16)
    wo = np.asarray(wo, dtype=np.float32).astype(bf16)
    # tri[k, q] = 1 if k <= q else 0   (causal keep-mask for diagonal blocks)
    tri = np.tril(np.ones((P, P), dtype=np.float32)).T.astype(bf16)
    vones = np.ones((P, NT * HG), dtype=bf16)
    xTs = [np.ascontiguousarray(x[b].T).astype(bf16) for b in range(B)]
    in_maps = []
    for core in range(NCORES):
        b, g = core // HG, core % HG
        jsl = slice(g * JW, (g + 1) * JW)
        in_maps.append({
            "xT": xTs[b],
            "wq": np.ascontiguousarray(wq[:, jsl]),
            "wk": np.ascontiguousarray(wk[:, jsl]),
            "wv": np.ascontiguousarray(wv[:, jsl]),
            "wo": np.ascontiguousarray(wo[jsl, :]),
            "tri": tri,
            "vones": vones,
        })
    return in_maps


def combine_outputs(results, bo):
    bo = np.asarray(bo, dtype=np.float32)
    y = np.zeros((B, S, D), dtype=np.float32)
    for core in range(NCORES):
        y[core // HG] += results[core]["y"]
    y += bo[None, None, :]
    return y


def kernel(x, wq, wk, wv, wo, bo):
    from concourse.bass_utils import run_bass_kernel_spmd

    nc = get_nc()
    in_maps = make_in_maps(x, wq, wk, wv, wo)
    res = run_bass_kernel_spmd(nc, in_maps, core_ids=list(range(NCORES)))
    return combine_outputs(res.results, bo)


# revision 30
# speedup vs baseline: 1.0398x; 1.0398x over previous
"""Trainium2 Bass kernel for causal multi-head attention.

Problem: B=2, S=2048, D=1024, H=16 heads (head_dim=64), fp32.
  y = softmax(causal(x@wq @ (x@wk)^T / sqrt(64))) @ (x@wv) @ wo + bo

Sharding (8 NeuronCores): 2 batches x 4 head-groups (4 heads each).
Each core computes, for its batch b and its 4 heads:
  - Q^T, K^T in [j, t] layout and V in [t, j] layout (j = 256 head cols)
  - scores^T[k, q] = K^T.T-free matmul, exp (scale=1/8, no max-sub --
    scores are ~N(0,1) so fp32 exp is safe), causal mask, then
    ctx^T[hd, q] with an appended ones-column giving softmax sums for free
  - per-q normalization via reciprocal + gpsimd partition-broadcast
  - partial y = ctx^T.T @ wo_slice  (row-shard of wo)
Host sums the 4 partials per batch and adds bo.

Matmul operands are bf16 (host-cast); accumulation is fp32 in PSUM.
"""

import numpy as np

B, S, D, H = 2, 2048, 1024, 16
HD = 64          # head dim
NCORES = 8
HG = 4           # heads per core
JW = HG * HD     # 256: per-core head columns
P = 128
DC = D // P      # 8 contraction chunks for projections
TCB = S // 512   # 4: 512-token blocks
NT = S // P      # 16: 128-token chunks

_CACHE = {}


def _build_nc(debug=False):
    import concourse.tile as tile
    from concourse import bacc, mybir

    f32 = mybir.dt.float32
    f32r = mybir.dt.bfloat16  # matmul operand dtype (fp32 accum in PSUM)
    EXP = mybir.ActivationFunctionType.Exp

    nc = bacc.Bacc(None, target_bir_lowering=False)

    xT_h = nc.dram_tensor("xT", [D, S], f32r, kind="ExternalInput")
    wq_h = nc.dram_tensor("wq", [D, JW], f32r, kind="ExternalInput")
    wk_h = nc.dram_tensor("wk", [D, JW], f32r, kind="ExternalInput")
    wv_h = nc.dram_tensor("wv", [D, JW], f32r, kind="ExternalInput")
    wo_h = nc.dram_tensor("wo", [JW, D], f32r, kind="ExternalInput")
    tri_h = nc.dram_tensor("tri", [P, P], f32r, kind="ExternalInput")
    vones_h = nc.dram_tensor("vones", [P, NT * HG], f32r, kind="ExternalInput")
    y_h = nc.dram_tensor("y", [S, D], f32, kind="ExternalOutput")
    if debug:
        dQT_h = nc.dram_tensor("dQT", [P, 2, S], f32r, kind="ExternalOutput")
        dKT_h = nc.dram_tensor("dKT", [P, 2, S], f32r, kind="ExternalOutput")
        dVg_h = nc.dram_tensor("dVg", [P, NT, HG * (HD + 2)], f32r,
                               kind="ExternalOutput")
        dctxn_h = nc.dram_tensor("dctxn", [P, 2, S], f32r, kind="ExternalOutput")
        dpc_h = nc.dram_tensor("dpc", [2, HD + 1, 512], f32, kind="ExternalOutput")
        drt_h = nc.dram_tensor("drt", [2, 512], f32, kind="ExternalOutput")
        drbc_h = nc.dram_tensor("drbc", [2, HD, 512], f32, kind="ExternalOutput")
        det_h = nc.dram_tensor("det", [4, P, 512], f32r, kind="ExternalOutput")

    with tile.TileContext(nc) as tc:
        with (
            tc.tile_pool(name="const", bufs=1) as cp,
            tc.tile_pool(name="work", bufs=2) as wp,
            tc.tile_pool(name="psum", bufs=2, space="PSUM") as pp,
        ):
            # ---- resident SBUF tensors ----
            xT_sb = cp.tile([P, DC, S], f32r, name="xT_sb")          # 64KB/part
            wq_sb = cp.tile([P, DC, JW], f32r, name="wq_sb")         # 8KB
            wk_sb = cp.tile([P, DC, JW], f32r, name="wk_sb")
            wv_sb = cp.tile([P, DC, JW], f32r, name="wv_sb")
            wo_sb = cp.tile([P, 2, D], f32r, name="wo_sb")           # 8KB
            tri_sb = cp.tile([P, P], f32r, name="tri_sb")
            QT = cp.tile([P, 2, S], f32r, name="QT")                 # 16KB
            KT = cp.tile([P, 2, S], f32r, name="KT")
            Vg = cp.tile([P, NT, HG * (HD + 2)], f32r, name="Vg")
            ctxn = cp.tile([P, 2, S], f32r, name="ctxn")             # 16KB

            # ---- input DMAs (small weights first so PE can start) ----
            nc.sync.dma_start(
                out=wk_sb, in_=wk_h[:].rearrange("(dc p) j -> p dc j", p=P)
            )
            nc.sync.dma_start(
                out=wq_sb, in_=wq_h[:].rearrange("(dc p) j -> p dc j", p=P)
            )
            for dc in range(DC):
                nc.sync.dma_start(
                    out=xT_sb[:, dc, :], in_=xT_h[dc * P:(dc + 1) * P, :]
                )
            nc.sync.dma_start(
                out=wv_sb, in_=wv_h[:].rearrange("(dc p) j -> p dc j", p=P)
            )
            nc.sync.dma_start(
                out=wo_sb, in_=wo_h[:].rearrange("(ch p) n -> p ch n", p=P)
            )
            nc.sync.dma_start(out=tri_sb, in_=tri_h[:, :])
            # ones columns of Vg (V writes fill the rest)
            nc.sync.dma_start(
                out=Vg[:, :, :].rearrange("p t (h c) -> p t h c", c=HD + 2)[:, :, :, HD],
                in_=vones_h[:].rearrange("p (t h) -> p t h", h=HG),
            )

            # ---- fused per-q-block pipeline:
            #      project this block's Q/K/V stripe, then attention ----
            for j4 in range(TCB):          # q-block [512*j4, 512*j4+512)
                tb = j4
                for w_sb, dst in ((wk_sb, KT), (wq_sb, QT)):
                    for jc in range(2):
                        pq = pp.tile([P, 512], f32, tag="mm", bufs=2, name="pq")
                        for dc in range(DC):
                            nc.tensor.matmul(
                                pq,
                                lhsT=w_sb[:, dc, jc * P:(jc + 1) * P],
                                rhs=xT_sb[:, dc, tb * 512:(tb + 1) * 512],
                                start=(dc == 0),
                                stop=(dc == DC - 1),
                            )
                        nc.vector.tensor_copy(
                            out=dst[:, jc, tb * 512:(tb + 1) * 512], in_=pq
                        )
                for tv in range(4 * j4, 4 * j4 + 4):
                    pv = pp.tile([P, JW], f32, tag="pcx", bufs=4, name="pv")
                    for dc in range(DC):
                        nc.tensor.matmul(
                            pv,
                            lhsT=xT_sb[:, dc, tv * P:(tv + 1) * P],
                            rhs=wv_sb[:, dc, :],
                            start=(dc == 0),
                            stop=(dc == DC - 1),
                        )
                    nc.vector.tensor_copy(
                        out=Vg[:, tv, :].rearrange("p (h c) -> p h c", c=HD + 2)[:, :, 0:HD],
                        in_=pv.rearrange("p (h c) -> p h c", c=HD),
                    )
                for pr in range(2):        # pair index = chunk index (h//2)
                    qs = slice(j4 * 512, (j4 + 1) * 512)
                    nchunks = 4 * j4 + 4
                    pctx = []
                    for hh in range(2):
                        pc = pp.tile([HD + 1, 512], f32, tag="pcx", bufs=4,
                                     name=f"pc{hh}")
                        pctx.append(pc)
                    for c in range(nchunks):
                        # columns [0, o) are fully masked for this k-chunk:
                        # skip them in scores, exp and ctx entirely.
                        o = P * (c - 4 * j4) if c >= 4 * j4 else 0
                        ps2 = pp.tile([P, 2, 512], f32, tag="mm", bufs=2,
                                      name="ps2")
                        for hh in range(2):
                            bp = HD * hh   # partition base for this head
                            nc.tensor.matmul(
                                ps2[:, hh, o:512],
                                lhsT=KT[bp:bp + HD, pr, c * P:(c + 1) * P],
                                rhs=QT[bp:bp + HD, pr,
                                       j4 * 512 + o:(j4 + 1) * 512],
                                start=True,
                                stop=True,
                            )
                        et = wp.tile([P, 2, 512], f32r, tag="exp", bufs=8,
                                     name="et")
                        nc.scalar.activation(
                            out=et[:, :, o:512], in_=ps2[:, :, o:512],
                            func=EXP, scale=0.125,
                        )
                        if c >= 4 * j4:
                            nc.vector.tensor_mul(
                                out=et[:, :, o:o + P],
                                in0=et[:, :, o:o + P],
                                in1=tri_sb[:, None, :].to_broadcast([P, 2, P]),
                            )
                        if debug and pr == 0 and j4 == 0:
                            nc.sync.dma_start(out=det_h[c], in_=et[:, 0, :])
                        for hh in range(2):
                            h = 2 * pr + hh
                            nc.tensor.matmul(
                                pctx[hh][:, o:512],
                                lhsT=Vg[:, c, h * (HD + 2):h * (HD + 2) + HD + 1],
                                rhs=et[:, hh, o:512],
                                start=(c == 0),
                                stop=(c == nchunks - 1),
                            )
                    # evacuate ctx psum early (frees the bank), then
                    # normalize: ctx^T[hd, q] * (1/sum[q])
                    for hh in range(2):
                        pc = pctx[hh]
                        # HW quirk: custom-DVE + partition_broadcast misread
                        # sources at partition base 64 -- hop sums to base 0.
                        sums = wp.tile([1, 512], f32, tag="sums", bufs=4,
                                       name="sums")
                        nc.vector.tensor_copy(out=sums, in_=pc[HD:HD + 1, :])
                        if debug and pr == 0 and j4 == 0:
                            dtmp = wp.tile([HD, 512], f32, tag="dtmp",
                                           bufs=2, name="dtmp")
                            nc.vector.tensor_copy(out=dtmp, in_=pc[0:HD, :])
                            nc.sync.dma_start(out=dpc_h[hh, 0:HD, :], in_=dtmp)
                            nc.sync.dma_start(out=dpc_h[hh, HD:HD + 1, :],
                                              in_=sums)
                        rt = wp.tile([1, 512], f32, tag="rt", bufs=4,
                                     name="rt")
                        nc.vector.reciprocal_approx_fast(out=rt, in_=sums)
                        rbc = wp.tile([HD, 512], f32, tag="rbc", bufs=4,
                                      name="rbc")
                        nc.gpsimd.partition_broadcast(
                            rbc[:, :], rt[0:1, :], channels=HD
                        )
                        if debug and pr == 0 and j4 == 0:
                            nc.sync.dma_start(out=drt_h[hh:hh + 1, :],
                                              in_=rt[0:1, :])
                            nc.sync.dma_start(out=drbc_h[hh], in_=rbc[:, :])
                        nc.vector.tensor_mul(
                            out=ctxn[HD * hh:HD * (hh + 1), pr, qs],
                            in0=pc[0:HD, :],
                            in1=rbc[:, :],
                        )

                # ---- output projection for this q-block ----
                for tb4 in range(4):
                    tb = 4 * j4 + tb4
                    ysb = wp.tile([P, D], f32, tag="y", bufs=2, name="ysb")
                    for nn in range(2):
                        py = pp.tile([P, 512], f32, tag="pcx", bufs=4,
                                     name="py")
                        for jc in range(2):
                            nc.tensor.matmul(
                                py,
                                lhsT=ctxn[:, jc, tb * P:(tb + 1) * P],
                                rhs=wo_sb[:, jc, nn * 512:(nn + 1) * 512],
                                start=(jc == 0),
                                stop=(jc == 1),
                            )
                        nc.scalar.copy(
                            out=ysb[:, nn * 512:(nn + 1) * 512], in_=py
                        )
                    nc.sync.dma_start(
                        out=y_h[tb * P:(tb + 1) * P, :], in_=ysb
                    )

            if debug:
                nc.sync.dma_start(out=dQT_h[:], in_=QT[:, :, :])
                nc.sync.dma_start(out=dKT_h[:], in_=KT[:, :, :])
                nc.sync.dma_start(out=dVg_h[:], in_=Vg[:, :, :])
                nc.sync.dma_start(out=dctxn_h[:], in_=ctxn[:, :, :])

    nc.compile()
    return nc


def get_nc(debug=False):
    key = ("nc", debug)
    if key not in _CACHE:
        _CACHE[key] = _build_nc(debug=debug)
    return _CACHE[key]


def make_in_maps(x, wq, wk, wv, wo):
    import ml_dtypes
    bf16 = ml_dtypes.bfloat16
    x = np.asarray(x, dtype=np.float32)
    wq = np.asarray(wq, dtype=np.float32).astype(bf16)
    wk = np.asarray(wk, dtype=np.float32).astype(bf16)
    wv = np.asarray(wv, dtype=np.float32).astype(bf16)
    wo = np.asarray(wo, dtype=np.float32).astype(bf16)
    # tri[k, q] = 1 if k <= q else 0   (causal keep-mask for diagonal blocks)
    tri = np.tril(np.ones((P, P), dtype=np.float32)).T.astype(bf16)
    vones = np.ones((P, NT * HG), dtype=bf16)
    xTs = [np.ascontiguousarray(x[b].T).astype(bf16) for b in range(B)]
    in_maps = []
    for core in range(NCORES):
        b, g = core // HG, core % HG
        jsl = slice(g * JW, (g + 1) * JW)
        in_maps.append({
            "xT": xTs[b],
            "wq": np.ascontiguousarray(wq[:, jsl]),
            "wk": np.ascontiguousarray(wk[:, jsl]),
            "wv": np.ascontiguousarray(wv[:, jsl]),
            "wo": np.ascontiguousarray(wo[jsl, :]),
            "tri": tri,
            "vones": vones,
        })
    return in_maps


def combine_outputs(results, bo):
    bo = np.asarray(bo, dtype=np.float32)
    y = np.zeros((B, S, D), dtype=np.float32)
    for core in range(NCORES):
        y[core // HG] += results[core]["y"]
    y += bo[None, None, :]
    return y


def kernel(x, wq, wk, wv, wo, bo):
    from concourse.bass_utils import run_bass_kernel_spmd

    nc = get_nc()
    in_maps = make_in_maps(x, wq, wk, wv, wo)
    res = run_bass_kernel_spmd(nc, in_maps, core_ids=list(range(NCORES)))
    return combine_outputs(res.results, bo)
